# revision 1
# baseline (speedup 1.0000x reference)
"""GQA attention kernel for 8 Trainium2 NeuronCores.

Sharding: core c = 4*b + h handles batch b (of 2) and kv-head h (of 4),
i.e. one kv head + its 4 grouped query heads. Each core computes its head
group's partial contribution to the output projection; the host sums the
4 partials per batch. No collectives.

Device math per core (S=2048, H=2048, d=128):
  QT_g = (x @ Wq_g + bq_g)^T          [d, S]   g=0..3   (fp32r matmuls)
  KT   = (x @ Wk_h)^T                 [d, S]            (bk cancels in softmax)
  V    = x @ Wv_h                     [S, d]   (computed as V^T then PE-transposed)
  S^T  = KT^T-blocks @ QT             [Sk, Sq] (bf16)
  P^T  = exp(SCALE * S^T)             (bf16, no max-subtraction: |s| <~ 6)
  y^T  = V^T-blocks.T @ P^T (PSUM accum), den = ones.T @ P^T (PE ones-matmul)
  yT  := y^T * (1/den broadcast via gpsimd partition_broadcast)
  out += yT_g^T @ Wo_g                [S, H]  (bf16, partial over this head group)
Host: out[b] = sum_h partial + (bv_rep @ Wo + bo).
"""

import numpy as np
import ml_dtypes

B = 2
S = 2048
HIDDEN = 2048
NKV = 4
GROUP = 4
D = 128
SCALE = D ** -0.5

BAND = 256            # S-columns per projection band
NBAND = S // BAND     # 8
NCH = HIDDEN // 128   # 16 contraction chunks
QTILE = 512           # queries per attention tile
NQT = S // QTILE      # 4
NSK = S // 128        # 16 key tiles

_CACHE = {}
LAST_RESULTS = None
TRACE = False
TMPDIR = None


def _build():
    import concourse.bass as bass
    import concourse.bacc as bacc
    import concourse.mybir as mybir
    import concourse.tile as tile
    from concourse.masks import make_identity

    f32 = mybir.dt.float32
    f32r = mybir.dt.float32r
    bf16 = mybir.dt.bfloat16
    EXP = mybir.ActivationFunctionType.Exp
    IDENT = mybir.ActivationFunctionType.Identity
    COPY = mybir.ActivationFunctionType.Copy

    nc = bacc.Bacc(trn_type="TRN2", target_bir_lowering=False, debug=False)

    xT = nc.dram_tensor("xT", [NBAND, 128, NCH, BAND], f32r, kind="ExternalInput").ap()
    wq = nc.dram_tensor("wq", [4, 128, NCH, 128], f32r, kind="ExternalInput").ap()
    wk = nc.dram_tensor("wk", [128, NCH, 128], f32r, kind="ExternalInput").ap()
    wv = nc.dram_tensor("wv", [128, NCH, 128], f32r, kind="ExternalInput").ap()
    wo = nc.dram_tensor("wo", [GROUP, 128, HIDDEN], bf16, kind="ExternalInput").ap()
    bq = nc.dram_tensor("bq", [128, GROUP], f32, kind="ExternalInput").ap()
    onesk = nc.dram_tensor("onesk", [128, 1], bf16, kind="ExternalInput").ap()
    out = nc.dram_tensor("out", [S, HIDDEN], f32, kind="ExternalOutput").ap()

    with tile.TileContext(nc) as tc:
        with (
            tc.tile_pool(name="const", bufs=1) as constp,
            tc.tile_pool(name="wts", bufs=1) as wtsp,
            tc.tile_pool(name="xb", bufs=2) as xbp,
            tc.tile_pool(name="qkv", bufs=1) as qkvp,
            tc.tile_pool(name="ptbuf", bufs=4) as ptp,
            tc.tile_pool(name="dens", bufs=3) as densp,
            tc.tile_pool(name="ytbuf", bufs=8) as ytp,
            tc.tile_pool(name="outbuf", bufs=2) as outp,
        ):
            # ---- DMAs in consumption order: consts, wk, band0 (in loop), wv, wq, wo ----
            onesk_t = constp.tile([128, 1], bf16, name="onesk_t")
            nc.sync.dma_start(out=onesk_t[:, :], in_=onesk)
            bq_t = constp.tile([128, GROUP], f32, name="bq_t")
            nc.sync.dma_start(out=bq_t[:, :], in_=bq)
            ident = constp.tile([128, 128], f32, name="ident")
            make_identity(nc, ident[:, :])

            wk_t = wtsp.tile([128, NCH, 128], f32r, name="wk_t")
            nc.sync.dma_start(out=wk_t[:, :, :], in_=wk)

            # band 0 load issued before the remaining weights
            bands = [None] * NBAND
            bands[0] = xbp.tile([128, NCH, BAND], f32r, name="band", tag="band")
            nc.sync.dma_start(out=bands[0][:, :, :], in_=xT[0])

            wv_t = wtsp.tile([128, NCH, 128], f32r, name="wv_t")
            nc.sync.dma_start(out=wv_t[:, :, :], in_=wv)
            wq_t = []
            for g in range(GROUP):
                t = wtsp.tile([128, NCH, 128], f32r, name=f"wq_t{g}", tag=f"wq{g}")
                nc.sync.dma_start(out=t[:, :, :], in_=wq[g])
                wq_t.append(t)

            # ---- persistent activations ----
            qt_t = []
            for g in range(GROUP):
                t = qkvp.tile([128, S], bf16, name=f"qt{g}", tag=f"qt{g}")
                qt_t.append(t)
            kt_t = qkvp.tile([128, S], bf16, name="kt_t")
            v_t = qkvp.tile([128, NSK * 128], bf16, name="v_t")
            vt_f = qkvp.tile([128, S], f32, name="vt_f")

            # =============== phase 1: projections ===============
            with tc.tile_pool(name="psA", bufs=1, space="PSUM") as psA:
                for b in range(NBAND):
                    if bands[b] is None:
                        bands[b] = xbp.tile(
                            [128, NCH, BAND], f32r, name="band", tag="band"
                        )
                        nc.sync.dma_start(out=bands[b][:, :, :], in_=xT[b])
                    band = bands[b]
                    bsl = slice(b * BAND, (b + 1) * BAND)

                    # K^T accumulation
                    pk = psA.tile([128, BAND], f32, name="pk", tag="pacc", bufs=3)
                    for c in range(NCH):
                        nc.tensor.matmul(
                            out=pk[:, :],
                            lhsT=wk_t[:, c, :],
                            rhs=band[:, c, :],
                            start=(c == 0), stop=(c == NCH - 1),
                        )
                    nc.scalar.activation(kt_t[:, bsl], pk[:, :], COPY)

                    # V^T accumulation (f32, transposed to V per 128-block later)
                    pv = psA.tile([128, BAND], f32, name="pv", tag="pacc", bufs=3)
                    for c in range(NCH):
                        nc.tensor.matmul(
                            out=pv[:, :],
                            lhsT=wv_t[:, c, :],
                            rhs=band[:, c, :],
                            start=(c == 0), stop=(c == NCH - 1),
                        )
                    nc.scalar.activation(vt_f[:, bsl], pv[:, :], COPY)

                    # Q^T per local head
                    for g in range(GROUP):
                        pq = psA.tile([128, BAND], f32, name="pq", tag="pacc", bufs=3)
                        for c in range(NCH):
                            nc.tensor.matmul(
                                out=pq[:, :],
                                lhsT=wq_t[g][:, c, :],
                                rhs=band[:, c, :],
                                start=(c == 0), stop=(c == NCH - 1),
                            )
                        nc.scalar.activation(
                            qt_t[g][:, bsl], pq[:, :], IDENT,
                            bias=bq_t[:, g:g + 1],
                        )

                    # transpose V^T band -> V (2 sk-tiles per band)
                    for t in range(BAND // 128):
                        sk = b * (BAND // 128) + t
                        pt = psA.tile([128, 128], f32, name="ptr", tag="pacc", bufs=3)
                        nc.tensor.transpose(
                            pt[:, :], vt_f[:, sk * 128:(sk + 1) * 128], ident[:, :]
                        )
                        nc.scalar.activation(
                            v_t[:, sk * 128:(sk + 1) * 128], pt[:, :], COPY
                        )

            # wo loads (needed only by out-projection, keep off the critical path)
            wo_t = []
            for g in range(GROUP):
                t = wtsp.tile([128, HIDDEN], bf16, name=f"wo_t{g}", tag=f"wo{g}")
                nc.sync.dma_start(out=t[:, :], in_=wo[g])
                wo_t.append(t)

            # =============== phase 2+3: attention + out-projection ===============
            with tc.tile_pool(name="psB", bufs=1, space="PSUM") as psB:
                yt_all = {}

                def attn_qtile(qt):
                    qsl = slice(qt * QTILE, (qt + 1) * QTILE)
                    for g in range(GROUP):
                        py = psB.tile([128, QTILE], f32, name="py", tag="yacc", bufs=2)
                        pden = psB.tile([1, QTILE], f32, name="pden", tag="den", bufs=2)
                        for skp in range(NSK // 2):
                            # paired sk tiles share one 2-bank scores tile and one exp
                            ps = psB.tile([128, 2 * QTILE], f32, name="ps", tag="sc", bufs=2)
                            for half in range(2):
                                sk = 2 * skp + half
                                nc.tensor.matmul(
                                    out=ps[:, half * QTILE:(half + 1) * QTILE],
                                    lhsT=kt_t[:, sk * 128:(sk + 1) * 128],
                                    rhs=qt_t[g][:, qsl],
                                    start=True, stop=True,
                                )
                            ptile = ptp.tile([128, 2 * QTILE], bf16, name="ptile", tag="pt")
                            nc.scalar.activation(ptile[:, :], ps[:, :], EXP, scale=SCALE)
                            for half in range(2):
                                sk = 2 * skp + half
                                hsl = slice(half * QTILE, (half + 1) * QTILE)
                                nc.tensor.matmul(
                                    out=py[:, :],
                                    lhsT=v_t[:, sk * 128:(sk + 1) * 128],
                                    rhs=ptile[:, hsl],
                                    start=(sk == 0), stop=(sk == NSK - 1),
                                )
                                nc.tensor.matmul(
                                    out=pden[:, :],
                                    lhsT=onesk_t[:, :],
                                    rhs=ptile[:, hsl],
                                    start=(sk == 0), stop=(sk == NSK - 1),
                                )
                        # normalize: yT = py * (1/den) broadcast over partitions
                        recip = densp.tile([1, QTILE], f32, name="recip", tag="recip")
                        nc.vector.reciprocal(recip[:, :], pden[:, :])
                        bcast = densp.tile([128, QTILE], f32, name="bcast", tag="bcast")
                        nc.gpsimd.partition_broadcast(bcast[:, :], recip[:, :])
                        yt = ytp.tile([128, QTILE], bf16, name="yt", tag="yt")
                        nc.vector.tensor_mul(yt[:, :], py[:, :], bcast[:, :])
                        yt_all[(qt, g)] = yt

                def outproj_qtile(qt):
                    for i in range(QTILE // 128):
                        outs = outp.tile([128, HIDDEN], f32, name="outs", tag="outs")
                        po = [
                            psB.tile([128, 2 * QTILE], f32, name=f"po{jp}", tag="sc", bufs=2)
                            for jp in range(2)
                        ]
                        for g in range(GROUP):
                            lhs = yt_all[(qt, g)][:, i * 128:(i + 1) * 128]
                            for j in range(4):
                                nc.tensor.matmul(
                                    out=po[j // 2][:, (j % 2) * 512:(j % 2 + 1) * 512],
                                    lhsT=lhs,
                                    rhs=wo_t[g][:, j * 512:(j + 1) * 512],
                                    start=(g == 0), stop=(g == GROUP - 1),
                                )
                        for jp in range(2):
                            nc.vector.tensor_copy(
                                outs[:, jp * 1024:(jp + 1) * 1024], po[jp][:, :]
                            )
                        r0 = qt * QTILE + i * 128
                        nc.sync.dma_start(out=out[r0:r0 + 128, :], in_=outs[:, :])

                for qt in range(NQT):
                    attn_qtile(qt)
                    outproj_qtile(qt)

    nc.finalize()
    return nc


def _get_nc():
    if "nc" not in _CACHE:
        _CACHE["nc"] = _build()
    return _CACHE["nc"]


def kernel(x, Wq, bq, Wk, bk, Wv, bv, Wo, bo):
    global LAST_RESULTS
    from concourse.bass_utils import run_bass_kernel_spmd

    x = np.asarray(x, np.float32)
    Wq = np.asarray(Wq, np.float32)
    Wk = np.asarray(Wk, np.float32)
    Wv = np.asarray(Wv, np.float32)
    Wo = np.asarray(Wo, np.float32)
    bq = np.asarray(bq, np.float32)
    bv = np.asarray(bv, np.float32)
    bo = np.asarray(bo, np.float32)

    nc = _get_nc()

    onesk_np = np.ones((128, 1), ml_dtypes.bfloat16)

    in_maps = []
    for c in range(8):
        b, h = divmod(c, NKV)
        xT = x[b].T  # [HIDDEN, S]
        xTh = np.ascontiguousarray(
            xT.reshape(NCH, 128, NBAND, BAND).transpose(2, 1, 0, 3)
        )
        # wq[g]: [128, NCH, 128] per local head
        wqh = np.ascontiguousarray(
            Wq[:, h * 512:(h + 1) * 512]
            .reshape(NCH, 128, GROUP, 128).transpose(2, 1, 0, 3)
        )
        wkh = np.ascontiguousarray(
            Wk[:, h * 128:(h + 1) * 128].reshape(NCH, 128, 128).transpose(1, 0, 2)
        )
        wvh = np.ascontiguousarray(
            Wv[:, h * 128:(h + 1) * 128].reshape(NCH, 128, 128).transpose(1, 0, 2)
        )
        woh = np.ascontiguousarray(
            Wo[h * 512:(h + 1) * 512, :].reshape(GROUP, 128, HIDDEN)
        ).astype(ml_dtypes.bfloat16)
        bqh = np.ascontiguousarray(
            bq[h * 512:(h + 1) * 512].reshape(GROUP, 128).T
        )
        in_maps.append({
            "xT": xTh, "wq": wqh, "wk": wkh, "wv": wvh, "wo": woh,
            "bq": bqh, "onesk": onesk_np,
        })

    res = run_bass_kernel_spmd(
        nc, in_maps, list(range(8)), trace=TRACE, tmpdir=TMPDIR
    )
    LAST_RESULTS = res

    # host-side constant bias: (bv repeated per head group) @ Wo + bo
    bv_rep = np.broadcast_to(
        bv.reshape(NKV, 1, D), (NKV, GROUP, D)
    ).reshape(HIDDEN)
    bias_row = bv_rep @ Wo + bo  # [HIDDEN]

    out = np.empty((B, S, HIDDEN), np.float32)
    for b in range(B):
        acc = res.results[b * NKV + 0]["out"].astype(np.float32)
        for h in range(1, NKV):
            acc = acc + res.results[b * NKV + h]["out"]
        out[b] = acc + bias_row
    return out



# revision 2
# speedup vs baseline: 1.0523x; 1.0523x over previous
"""GQA attention kernel for 8 Trainium2 NeuronCores (v2).

Sharding: core c = 4*b + h handles batch b (of 2) and kv-head h (of 4),
i.e. one kv head + its 4 grouped query heads. Each core computes its head
group's partial contribution to the output projection; the host sums the
4 partials per batch. No collectives.

v2 changes vs v1 (461us):
  - all inputs bf16 (halves DMA, full-rate matmuls), BAND=512 projections
  - softmax denominator via vector accumulation of P tiles + gpsimd
    partition_all_reduce -- no PE den matmuls (-18% PE stream), no slow
    [1,512] reciprocal (3.3us each)
  - attention inner loop software-pipelined: scores(sk+1) issued between
    PV(sk) matmuls so the exp latency never stalls the PE
  - QTILE=1024 (fewer, longer instruction groups), out-projection of
    qtile 0 interleaved between attention g-blocks of qtile 1
  - output partials in bf16 (halves output DMA)

Device math per core (S=2048, H=2048, d=128):
  QT_g = (x @ Wq_g + bq_g)^T          [d, S]   g=0..3   (bf16 matmuls)
  KT   = (x @ Wk_h)^T                 [d, S]            (bk cancels in softmax)
  V    = x @ Wv_h                     [S, d]   (V^T then PE-transposed)
  S^T  = KT^T-blocks @ QT             [Sk, Sq]
  P^T  = exp(SCALE * S^T)             (bf16, no max-subtraction: |s| <~ 5)
  y^T  = V^T-blocks.T @ P^T (PSUM accum); den = partition_all_reduce(sum P^T)
  yT  := y^T * recip(den)
  out += yT_g^T @ Wo_g                [S, H]  partial, bf16 to HBM
Host: out[b] = sum_h partial + (bv_rep @ Wo + bo).
"""

import numpy as np
import ml_dtypes

B = 2
S = 2048
HIDDEN = 2048
NKV = 4
GROUP = 4
D = 128
SCALE = D ** -0.5

BAND = 512            # S-columns per projection band
NBAND = S // BAND     # 4
NCH = HIDDEN // 128   # 16 contraction chunks
QT2 = 1024            # queries per attention tile
NQT2 = S // QT2       # 2
NSK = S // 128        # 16 key tiles

_CACHE = {}
LAST_RESULTS = None
TRACE = False
TMPDIR = None


def _build():
    import concourse.bass as bass
    import concourse.bacc as bacc
    import concourse.mybir as mybir
    import concourse.tile as tile
    import concourse.bass_isa as bass_isa
    from concourse.masks import make_identity

    f32 = mybir.dt.float32
    bf16 = mybir.dt.bfloat16
    EXP = mybir.ActivationFunctionType.Exp
    IDENT = mybir.ActivationFunctionType.Identity
    COPY = mybir.ActivationFunctionType.Copy

    nc = bacc.Bacc(trn_type="TRN2", target_bir_lowering=False, debug=False)

    xT = nc.dram_tensor("xT", [NBAND, 128, NCH, BAND], bf16, kind="ExternalInput").ap()
    wq = nc.dram_tensor("wq", [GROUP, 128, NCH, 128], bf16, kind="ExternalInput").ap()
    wk = nc.dram_tensor("wk", [128, NCH, 128], bf16, kind="ExternalInput").ap()
    wv = nc.dram_tensor("wv", [128, NCH, 128], bf16, kind="ExternalInput").ap()
    wo = nc.dram_tensor("wo", [GROUP, 128, HIDDEN], bf16, kind="ExternalInput").ap()
    bq = nc.dram_tensor("bq", [128, GROUP], f32, kind="ExternalInput").ap()
    out = nc.dram_tensor("out", [S, HIDDEN], bf16, kind="ExternalOutput").ap()

    with tile.TileContext(nc) as tc:
        with (
            tc.tile_pool(name="const", bufs=1) as constp,
            tc.tile_pool(name="wts", bufs=1) as wtsp,
            tc.tile_pool(name="xb", bufs=2) as xbp,
            tc.tile_pool(name="qkv", bufs=1) as qkvp,
            tc.tile_pool(name="ptbuf", bufs=3) as ptp,
            tc.tile_pool(name="accb", bufs=2) as accp,
            tc.tile_pool(name="dens", bufs=2) as densp,
            tc.tile_pool(name="ytbuf", bufs=8) as ytp,
            tc.tile_pool(name="outbuf", bufs=2) as outp,
        ):
            # ---- DMAs in consumption order: consts, wk, band0, wv, wq ----
            bq_t = constp.tile([128, GROUP], f32, name="bq_t")
            nc.sync.dma_start(out=bq_t[:, :], in_=bq)
            ident = constp.tile([128, 128], f32, name="ident")
            make_identity(nc, ident[:, :])

            wk_t = wtsp.tile([128, NCH, 128], bf16, name="wk_t")
            nc.sync.dma_start(out=wk_t[:, :, :], in_=wk)

            # band 0 load split in two chunk-halves so the first K matmuls
            # can start before the whole band lands
            bands = [None] * NBAND
            bands[0] = xbp.tile([128, NCH, BAND], bf16, name="band", tag="band")
            nc.sync.dma_start(out=bands[0][:, 0:8, :], in_=xT[0, :, 0:8, :])
            nc.sync.dma_start(out=bands[0][:, 8:16, :], in_=xT[0, :, 8:16, :])

            wv_t = wtsp.tile([128, NCH, 128], bf16, name="wv_t")
            nc.sync.dma_start(out=wv_t[:, :, :], in_=wv)
            wq_t = []
            for g in range(GROUP):
                t = wtsp.tile([128, NCH, 128], bf16, name=f"wq_t{g}", tag=f"wq{g}")
                nc.sync.dma_start(out=t[:, :, :], in_=wq[g])
                wq_t.append(t)

            # ---- persistent activations ----
            qt_t = []
            for g in range(GROUP):
                t = qkvp.tile([128, S], bf16, name=f"qt{g}", tag=f"qt{g}")
                qt_t.append(t)
            kt_t = qkvp.tile([128, S], bf16, name="kt_t")
            v_t = qkvp.tile([128, NSK * 128], bf16, name="v_t")
            vt_f = qkvp.tile([128, S], f32, name="vt_f")

            # =============== phase 1: projections ===============
            with tc.tile_pool(name="psA", bufs=1, space="PSUM") as psA:
                for b in range(NBAND):
                    if bands[b] is None:
                        bands[b] = xbp.tile(
                            [128, NCH, BAND], bf16, name="band", tag="band"
                        )
                        nc.sync.dma_start(out=bands[b][:, :, :], in_=xT[b])
                    band = bands[b]
                    bsl = slice(b * BAND, (b + 1) * BAND)

                    # K^T accumulation
                    pk = psA.tile([128, BAND], f32, name="pk", tag="pacc", bufs=4)
                    for c in range(NCH):
                        nc.tensor.matmul(
                            out=pk[:, :],
                            lhsT=wk_t[:, c, :],
                            rhs=band[:, c, :],
                            start=(c == 0), stop=(c == NCH - 1),
                        )
                    nc.scalar.activation(kt_t[:, bsl], pk[:, :], COPY)

                    # V^T accumulation (f32, transposed to V per 128-block later)
                    pv = psA.tile([128, BAND], f32, name="pv", tag="pacc", bufs=4)
                    for c in range(NCH):
                        nc.tensor.matmul(
                            out=pv[:, :],
                            lhsT=wv_t[:, c, :],
                            rhs=band[:, c, :],
                            start=(c == 0), stop=(c == NCH - 1),
                        )
                    nc.scalar.activation(vt_f[:, bsl], pv[:, :], COPY)

                    # Q^T per local head
                    for g in range(GROUP):
                        pq = psA.tile([128, BAND], f32, name="pq", tag="pacc", bufs=4)
                        for c in range(NCH):
                            nc.tensor.matmul(
                                out=pq[:, :],
                                lhsT=wq_t[g][:, c, :],
                                rhs=band[:, c, :],
                                start=(c == 0), stop=(c == NCH - 1),
                            )
                        nc.scalar.activation(
                            qt_t[g][:, bsl], pq[:, :], IDENT,
                            bias=bq_t[:, g:g + 1],
                        )

                    # transpose V^T band -> V (4 sk-tiles per band)
                    for t in range(BAND // 128):
                        sk = b * (BAND // 128) + t
                        pt = psA.tile([128, 128], f32, name="ptr", tag="pacc", bufs=4)
                        nc.tensor.transpose(
                            pt[:, :], vt_f[:, sk * 128:(sk + 1) * 128], ident[:, :]
                        )
                        nc.scalar.activation(
                            v_t[:, sk * 128:(sk + 1) * 128], pt[:, :], COPY
                        )

            # wo loads (needed only by out-projection, off the critical path)
            wo_t = []
            for g in range(GROUP):
                t = wtsp.tile([128, HIDDEN], bf16, name=f"wo_t{g}", tag=f"wo{g}")
                nc.sync.dma_start(out=t[:, :], in_=wo[g])
                wo_t.append(t)

            # =============== phase 2+3: attention + out-projection ===============
            with tc.tile_pool(name="psB", bufs=1, space="PSUM") as psB:
                yt_all = {}

                def attn_block(qt2, g):
                    """Attention for one head g over query tile qt2 (1024 q).

                    Inner loop software-pipelined: scores for sk+1 are issued
                    between the PV matmuls of sk so the PE never waits on exp.
                    """
                    q0 = qt2 * QT2
                    py = [
                        psB.tile([128, 512], f32, name=f"py{h}", tag="py", bufs=4)
                        for h in range(2)
                    ]
                    acc = accp.tile([128, QT2], f32, name="acc", tag="acc")
                    pts = [None] * NSK

                    def scores(sk, h):
                        ps = psB.tile([128, 512], f32, name="ps", tag="ps", bufs=2)
                        nc.tensor.matmul(
                            out=ps[:, :],
                            lhsT=kt_t[:, sk * 128:(sk + 1) * 128],
                            rhs=qt_t[g][:, q0 + h * 512: q0 + (h + 1) * 512],
                            start=True, stop=True,
                        )
                        if pts[sk] is None:
                            pts[sk] = ptp.tile([128, QT2], bf16, name="pt", tag="pt")
                        nc.scalar.activation(
                            pts[sk][:, h * 512:(h + 1) * 512], ps[:, :], EXP,
                            scale=SCALE,
                        )

                    scores(0, 0)
                    scores(0, 1)
                    for sk in range(NSK):
                        if sk + 1 < NSK:
                            scores(sk + 1, 0)
                        nc.tensor.matmul(
                            out=py[0][:, :],
                            lhsT=v_t[:, sk * 128:(sk + 1) * 128],
                            rhs=pts[sk][:, 0:512],
                            start=(sk == 0), stop=(sk == NSK - 1),
                        )
                        if sk + 1 < NSK:
                            scores(sk + 1, 1)
                        nc.tensor.matmul(
                            out=py[1][:, :],
                            lhsT=v_t[:, sk * 128:(sk + 1) * 128],
                            rhs=pts[sk][:, 512:1024],
                            start=(sk == 0), stop=(sk == NSK - 1),
                        )
                        if sk == 0:
                            nc.vector.tensor_copy(acc[:, :], pts[sk][:, :])
                        else:
                            nc.vector.tensor_add(acc[:, :], acc[:, :], pts[sk][:, :])

                    # denominator: all-partition sum of acc, broadcast to all
                    # partitions by the same gpsimd op; then recip + scale
                    bden = densp.tile([128, QT2], f32, name="bden", tag="bden")
                    nc.gpsimd.partition_all_reduce(
                        bden[:, :], acc[:, :], channels=128,
                        reduce_op=bass_isa.ReduceOp.add,
                    )
                    brecip = densp.tile([128, QT2], f32, name="brecip", tag="brecip")
                    nc.vector.reciprocal(brecip[:, :], bden[:, :])
                    yt = ytp.tile([128, QT2], bf16, name="yt", tag="yt")
                    for h in range(2):
                        nc.vector.tensor_mul(
                            yt[:, h * 512:(h + 1) * 512],
                            py[h][:, :],
                            brecip[:, h * 512:(h + 1) * 512],
                        )
                    yt_all[(qt2, g)] = yt

                def outproj(qt2):
                    for i in range(QT2 // 128):
                        outs = outp.tile([128, HIDDEN], bf16, name="outs", tag="outs")
                        for j in range(4):
                            po = psB.tile([128, 512], f32, name="po", tag="po", bufs=2)
                            for g in range(GROUP):
                                nc.tensor.matmul(
                                    out=po[:, :],
                                    lhsT=yt_all[(qt2, g)][:, i * 128:(i + 1) * 128],
                                    rhs=wo_t[g][:, j * 512:(j + 1) * 512],
                                    start=(g == 0), stop=(g == GROUP - 1),
                                )
                            # drains alternate engines to balance queues
                            if j % 2 == 0:
                                nc.scalar.activation(
                                    outs[:, j * 512:(j + 1) * 512], po[:, :], COPY
                                )
                            else:
                                nc.vector.tensor_copy(
                                    outs[:, j * 512:(j + 1) * 512], po[:, :]
                                )
                        r0 = qt2 * QT2 + i * 128
                        nc.sync.dma_start(out=out[r0:r0 + 128, :], in_=outs[:, :])

                for g in range(GROUP):
                    attn_block(0, g)
                attn_block(1, 0)
                outproj(0)
                for g in range(1, GROUP):
                    attn_block(1, g)
                outproj(1)

    nc.finalize()
    return nc


def _get_nc():
    if "nc" not in _CACHE:
        _CACHE["nc"] = _build()
    return _CACHE["nc"]


def kernel(x, Wq, bq, Wk, bk, Wv, bv, Wo, bo):
    global LAST_RESULTS
    from concourse.bass_utils import run_bass_kernel_spmd

    bf = ml_dtypes.bfloat16
    x = np.asarray(x, np.float32)
    Wq = np.asarray(Wq, np.float32)
    Wk = np.asarray(Wk, np.float32)
    Wv = np.asarray(Wv, np.float32)
    Wo = np.asarray(Wo, np.float32)
    bq = np.asarray(bq, np.float32)
    bv = np.asarray(bv, np.float32)
    bo = np.asarray(bo, np.float32)

    nc = _get_nc()

    in_maps = []
    for c in range(8):
        b, h = divmod(c, NKV)
        xT = x[b].T  # [HIDDEN, S]
        xTh = np.ascontiguousarray(
            xT.reshape(NCH, 128, NBAND, BAND).transpose(2, 1, 0, 3)
        ).astype(bf)
        # wq[g]: [128, NCH, 128] per local head
        wqh = np.ascontiguousarray(
            Wq[:, h * 512:(h + 1) * 512]
            .reshape(NCH, 128, GROUP, 128).transpose(2, 1, 0, 3)
        ).astype(bf)
        wkh = np.ascontiguousarray(
            Wk[:, h * 128:(h + 1) * 128].reshape(NCH, 128, 128).transpose(1, 0, 2)
        ).astype(bf)
        wvh = np.ascontiguousarray(
            Wv[:, h * 128:(h + 1) * 128].reshape(NCH, 128, 128).transpose(1, 0, 2)
        ).astype(bf)
        woh = np.ascontiguousarray(
            Wo[h * 512:(h + 1) * 512, :].reshape(GROUP, 128, HIDDEN)
        ).astype(bf)
        bqh = np.ascontiguousarray(
            bq[h * 512:(h + 1) * 512].reshape(GROUP, 128).T
        )
        in_maps.append({
            "xT": xTh, "wq": wqh, "wk": wkh, "wv": wvh, "wo": woh,
            "bq": bqh,
        })

    res = run_bass_kernel_spmd(
        nc, in_maps, list(range(8)), trace=TRACE, tmpdir=TMPDIR
    )
    LAST_RESULTS = res

    # host-side constant bias: (bv repeated per head group) @ Wo + bo
    bv_rep = np.broadcast_to(
        bv.reshape(NKV, 1, D), (NKV, GROUP, D)
    ).reshape(HIDDEN)
    bias_row = bv_rep @ Wo + bo  # [HIDDEN]

    out = np.empty((B, S, HIDDEN), np.float32)
    for b in range(B):
        acc = res.results[b * NKV + 0]["out"].astype(np.float32)
        for h in range(1, NKV):
            acc = acc + res.results[b * NKV + h]["out"].astype(np.float32)
        out[b] = acc + bias_row
    return out


# revision 4
# speedup vs baseline: 1.2256x; 1.1647x over previous
"""GQA attention kernel for 8 Trainium2 NeuronCores (v2).

Sharding: core c = 4*b + h handles batch b (of 2) and kv-head h (of 4),
i.e. one kv head + its 4 grouped query heads. Each core computes its head
group's partial contribution to the output projection; the host sums the
4 partials per batch. No collectives.

v2 changes vs v1 (461us):
  - all inputs bf16 (halves DMA, full-rate matmuls), BAND=512 projections
  - softmax denominator via vector accumulation of P tiles + gpsimd
    partition_all_reduce -- no PE den matmuls (-18% PE stream), no slow
    [1,512] reciprocal (3.3us each)
  - attention inner loop software-pipelined: scores(sk+1) issued between
    PV(sk) matmuls so the exp latency never stalls the PE
  - QTILE=1024 (fewer, longer instruction groups), out-projection of
    qtile 0 interleaved between attention g-blocks of qtile 1
  - output partials in bf16 (halves output DMA)

Device math per core (S=2048, H=2048, d=128):
  QT_g = (x @ Wq_g + bq_g)^T          [d, S]   g=0..3   (bf16 matmuls)
  KT   = (x @ Wk_h)^T                 [d, S]            (bk cancels in softmax)
  V    = x @ Wv_h                     [S, d]   (V^T then PE-transposed)
  S^T  = KT^T-blocks @ QT             [Sk, Sq]
  P^T  = exp(SCALE * S^T)             (bf16, no max-subtraction: |s| <~ 5)
  y^T  = V^T-blocks.T @ P^T (PSUM accum); den = partition_all_reduce(sum P^T)
  yT  := y^T * recip(den)
  out += yT_g^T @ Wo_g                [S, H]  partial, bf16 to HBM
Host: out[b] = sum_h partial + (bv_rep @ Wo + bo).
"""

import numpy as np
import ml_dtypes

B = 2
S = 2048
HIDDEN = 2048
NKV = 4
GROUP = 4
D = 128
SCALE = D ** -0.5

BAND = 512            # S-columns per projection band
NBAND = S // BAND     # 4
NCH = HIDDEN // 128   # 16 contraction chunks
QT2 = 1024            # queries per attention tile
NQT2 = S // QT2       # 2
NSK = S // 128        # 16 key tiles

_CACHE = {}
LAST_RESULTS = None
TRACE = False
TMPDIR = None


def _build():
    import concourse.bass as bass
    import concourse.bacc as bacc
    import concourse.mybir as mybir
    import concourse.tile as tile
    import concourse.bass_isa as bass_isa
    from concourse.masks import make_identity

    f32 = mybir.dt.float32
    bf16 = mybir.dt.bfloat16
    EXP = mybir.ActivationFunctionType.Exp
    IDENT = mybir.ActivationFunctionType.Identity
    COPY = mybir.ActivationFunctionType.Copy

    nc = bacc.Bacc(trn_type="TRN2", target_bir_lowering=False, debug=False)

    xT = nc.dram_tensor("xT", [NBAND, 128, NCH, BAND], bf16, kind="ExternalInput").ap()
    wq = nc.dram_tensor("wq", [GROUP, 128, NCH, 128], bf16, kind="ExternalInput").ap()
    wk = nc.dram_tensor("wk", [128, NCH, 128], bf16, kind="ExternalInput").ap()
    wv = nc.dram_tensor("wv", [128, NCH, 128], bf16, kind="ExternalInput").ap()
    wo = nc.dram_tensor("wo", [GROUP, 128, HIDDEN], bf16, kind="ExternalInput").ap()
    bq = nc.dram_tensor("bq", [128, GROUP], f32, kind="ExternalInput").ap()
    out = nc.dram_tensor("out", [S, HIDDEN], bf16, kind="ExternalOutput").ap()

    with tile.TileContext(nc) as tc:
        with (
            tc.tile_pool(name="const", bufs=1) as constp,
            tc.tile_pool(name="wts", bufs=1) as wtsp,
            tc.tile_pool(name="xb", bufs=2) as xbp,
            tc.tile_pool(name="qkv", bufs=1) as qkvp,
            tc.tile_pool(name="ptbuf", bufs=3) as ptp,
            tc.tile_pool(name="accb", bufs=2) as accp,
            tc.tile_pool(name="dens", bufs=2) as densp,
            tc.tile_pool(name="ytbuf", bufs=8) as ytp,
            tc.tile_pool(name="outbuf", bufs=2) as outp,
        ):
            # ---- DMAs in consumption order: consts, wk, band0, wv, wq ----
            bq_t = constp.tile([128, GROUP], f32, name="bq_t")
            nc.sync.dma_start(out=bq_t[:, :], in_=bq)
            ident = constp.tile([128, 128], f32, name="ident")
            make_identity(nc, ident[:, :])

            # wk and band 0 split into chunk-groups so the first K matmuls
            # can start before the whole tensors land
            wk_t = wtsp.tile([128, NCH, 128], bf16, name="wk_t")
            nc.sync.dma_start(out=wk_t[:, 0:4, :], in_=wk[:, 0:4, :])
            bands = [None] * NBAND
            bands[0] = xbp.tile([128, NCH, BAND], bf16, name="band", tag="band")
            nc.sync.dma_start(out=bands[0][:, 0:4, :], in_=xT[0, :, 0:4, :])
            nc.sync.dma_start(out=wk_t[:, 4:16, :], in_=wk[:, 4:16, :])
            for cq in range(1, 4):
                nc.sync.dma_start(
                    out=bands[0][:, 4 * cq:4 * (cq + 1), :],
                    in_=xT[0, :, 4 * cq:4 * (cq + 1), :],
                )

            wv_t = wtsp.tile([128, NCH, 128], bf16, name="wv_t")
            nc.sync.dma_start(out=wv_t[:, :, :], in_=wv)
            wq_t = []
            for g in range(GROUP):
                t = wtsp.tile([128, NCH, 128], bf16, name=f"wq_t{g}", tag=f"wq{g}")
                nc.sync.dma_start(out=t[:, :, :], in_=wq[g])
                wq_t.append(t)

            # ---- persistent activations ----
            qt_t = []
            for g in range(GROUP):
                t = qkvp.tile([128, S], bf16, name=f"qt{g}", tag=f"qt{g}")
                qt_t.append(t)
            kt_t = qkvp.tile([128, S], bf16, name="kt_t")
            v_t = qkvp.tile([128, NSK * 128], bf16, name="v_t")
            vt_f = qkvp.tile([128, S], f32, name="vt_f")

            # =============== phase 1: projections ===============
            with tc.tile_pool(name="psA", bufs=1, space="PSUM") as psA:
                for b in range(NBAND):
                    if bands[b] is None:
                        bands[b] = xbp.tile(
                            [128, NCH, BAND], bf16, name="band", tag="band"
                        )
                        nc.sync.dma_start(out=bands[b][:, :, :], in_=xT[b])
                    band = bands[b]
                    bsl = slice(b * BAND, (b + 1) * BAND)

                    # K^T accumulation
                    pk = psA.tile([128, BAND], f32, name="pk", tag="pacc", bufs=4)
                    for c in range(NCH):
                        nc.tensor.matmul(
                            out=pk[:, :],
                            lhsT=wk_t[:, c, :],
                            rhs=band[:, c, :],
                            start=(c == 0), stop=(c == NCH - 1),
                        )
                    nc.scalar.activation(kt_t[:, bsl], pk[:, :], COPY)

                    # V^T accumulation (f32, transposed to V per 128-block later)
                    pv = psA.tile([128, BAND], f32, name="pv", tag="pacc", bufs=4)
                    for c in range(NCH):
                        nc.tensor.matmul(
                            out=pv[:, :],
                            lhsT=wv_t[:, c, :],
                            rhs=band[:, c, :],
                            start=(c == 0), stop=(c == NCH - 1),
                        )
                    nc.scalar.activation(vt_f[:, bsl], pv[:, :], COPY)

                    # Q^T per local head
                    for g in range(GROUP):
                        pq = psA.tile([128, BAND], f32, name="pq", tag="pacc", bufs=4)
                        for c in range(NCH):
                            nc.tensor.matmul(
                                out=pq[:, :],
                                lhsT=wq_t[g][:, c, :],
                                rhs=band[:, c, :],
                                start=(c == 0), stop=(c == NCH - 1),
                            )
                        nc.scalar.activation(
                            qt_t[g][:, bsl], pq[:, :], IDENT,
                            bias=bq_t[:, g:g + 1],
                        )

                    # transpose V^T band -> V (4 sk-tiles per band)
                    for t in range(BAND // 128):
                        sk = b * (BAND // 128) + t
                        pt = psA.tile([128, 128], f32, name="ptr", tag="pacc", bufs=4)
                        nc.tensor.transpose(
                            pt[:, :], vt_f[:, sk * 128:(sk + 1) * 128], ident[:, :]
                        )
                        nc.scalar.activation(
                            v_t[:, sk * 128:(sk + 1) * 128], pt[:, :], COPY
                        )

            # wo loads (needed only by out-projection, off the critical path)
            wo_t = []
            for g in range(GROUP):
                t = wtsp.tile([128, HIDDEN], bf16, name=f"wo_t{g}", tag=f"wo{g}")
                nc.sync.dma_start(out=t[:, :], in_=wo[g])
                wo_t.append(t)

            # =============== phase 2+3: attention + out-projection ===============
            # PSUM budget (8 banks): tag "pp" [128,1024] bufs=2 (4 banks),
            # shared by score pairs and out-proj pairs (never live at once);
            # tag "py" [128,1024] bufs=2 (4 banks).
            with tc.tile_pool(name="psB", bufs=1, space="PSUM") as psB:
                yt_all = {}

                def attn_block(qt2, g):
                    """Attention for one head g over query tile qt2 (1024 q).

                    Inner loop software-pipelined: scores for sk+1 are issued
                    between the PV matmuls of sk so the PE never waits on exp.
                    One [128,1024] exp per sk (half the scalar-engine overhead
                    of two 512-wide ones).
                    """
                    q0 = qt2 * QT2
                    py = psB.tile([128, QT2], f32, name="py", tag="py", bufs=2)
                    acc = accp.tile([128, QT2], bf16, name="acc", tag="acc")
                    pts = [None] * NSK
                    pss = [None] * NSK

                    def scores(sk, h):
                        if h == 0:
                            pss[sk] = psB.tile(
                                [128, QT2], f32, name="ps", tag="pp", bufs=2
                            )
                        nc.tensor.matmul(
                            out=pss[sk][:, h * 512:(h + 1) * 512],
                            lhsT=kt_t[:, sk * 128:(sk + 1) * 128],
                            rhs=qt_t[g][:, q0 + h * 512: q0 + (h + 1) * 512],
                            start=True, stop=True,
                        )
                        if h == 1:
                            pts[sk] = ptp.tile([128, QT2], bf16, name="pt", tag="pt")
                            nc.scalar.activation(
                                pts[sk][:, :], pss[sk][:, :], EXP, scale=SCALE,
                            )

                    scores(0, 0)
                    scores(0, 1)
                    for sk in range(NSK):
                        if sk + 1 < NSK:
                            scores(sk + 1, 0)
                            scores(sk + 1, 1)
                        for h in range(2):
                            nc.tensor.matmul(
                                out=py[:, h * 512:(h + 1) * 512],
                                lhsT=v_t[:, sk * 128:(sk + 1) * 128],
                                rhs=pts[sk][:, h * 512:(h + 1) * 512],
                                start=(sk == 0), stop=(sk == NSK - 1),
                            )
                        if sk == 0:
                            nc.vector.tensor_copy(acc[:, :], pts[sk][:, :])
                        else:
                            nc.vector.tensor_add(acc[:, :], acc[:, :], pts[sk][:, :])

                    # denominator: all-partition sum of acc (bf16 in, f32 out),
                    # fast approximate reciprocal, then scale y^T
                    bden = densp.tile([128, QT2], f32, name="bden", tag="bden")
                    nc.gpsimd.partition_all_reduce(
                        bden[:, :], acc[:, :], channels=128,
                        reduce_op=bass_isa.ReduceOp.add,
                    )
                    brecip = densp.tile([128, QT2], f32, name="brecip", tag="brecip")
                    nc.vector.reciprocal_approx_fast(brecip[:, :], bden[:, :])
                    yt = ytp.tile([128, QT2], bf16, name="yt", tag="yt")
                    nc.vector.tensor_mul(yt[:, :], py[:, :], brecip[:, :])
                    yt_all[(qt2, g)] = yt

                def outproj_iblocks(qt2, iblocks):
                    for i in iblocks:
                        outs = outp.tile([128, HIDDEN], bf16, name="outs", tag="outs")
                        for jp in range(2):
                            po = psB.tile([128, QT2], f32, name="po", tag="pp", bufs=2)
                            for jj in range(2):
                                j = jp * 2 + jj
                                for g in range(GROUP):
                                    nc.tensor.matmul(
                                        out=po[:, jj * 512:(jj + 1) * 512],
                                        lhsT=yt_all[(qt2, g)][:, i * 128:(i + 1) * 128],
                                        rhs=wo_t[g][:, j * 512:(j + 1) * 512],
                                        start=(g == 0), stop=(g == GROUP - 1),
                                    )
                            # drains alternate engines to balance queues
                            if (i + jp) % 2 == 0:
                                nc.scalar.activation(
                                    outs[:, jp * 1024:(jp + 1) * 1024], po[:, :], COPY
                                )
                            else:
                                nc.vector.tensor_copy(
                                    outs[:, jp * 1024:(jp + 1) * 1024], po[:, :]
                                )
                        r0 = qt2 * QT2 + i * 128
                        nc.sync.dma_start(out=out[r0:r0 + 128, :], in_=outs[:, :])

                for g in range(GROUP):
                    attn_block(0, g)
                # out-proj of qtile 0 spread between qtile-1 attention blocks
                for g in range(GROUP):
                    attn_block(1, g)
                    outproj_iblocks(0, [2 * g, 2 * g + 1])
                outproj_iblocks(1, list(range(8)))

    nc.finalize()
    return nc


def _get_nc():
    if "nc" not in _CACHE:
        _CACHE["nc"] = _build()
    return _CACHE["nc"]


def kernel(x, Wq, bq, Wk, bk, Wv, bv, Wo, bo):
    global LAST_RESULTS
    from concourse.bass_utils import run_bass_kernel_spmd

    bf = ml_dtypes.bfloat16
    x = np.asarray(x, np.float32)
    Wq = np.asarray(Wq, np.float32)
    Wk = np.asarray(Wk, np.float32)
    Wv = np.asarray(Wv, np.float32)
    Wo = np.asarray(Wo, np.float32)
    bq = np.asarray(bq, np.float32)
    bv = np.asarray(bv, np.float32)
    bo = np.asarray(bo, np.float32)

    nc = _get_nc()

    in_maps = []
    for c in range(8):
        b, h = divmod(c, NKV)
        xT = x[b].T  # [HIDDEN, S]
        xTh = np.ascontiguousarray(
            xT.reshape(NCH, 128, NBAND, BAND).transpose(2, 1, 0, 3)
        ).astype(bf)
        # wq[g]: [128, NCH, 128] per local head
        wqh = np.ascontiguousarray(
            Wq[:, h * 512:(h + 1) * 512]
            .reshape(NCH, 128, GROUP, 128).transpose(2, 1, 0, 3)
        ).astype(bf)
        wkh = np.ascontiguousarray(
            Wk[:, h * 128:(h + 1) * 128].reshape(NCH, 128, 128).transpose(1, 0, 2)
        ).astype(bf)
        wvh = np.ascontiguousarray(
            Wv[:, h * 128:(h + 1) * 128].reshape(NCH, 128, 128).transpose(1, 0, 2)
        ).astype(bf)
        woh = np.ascontiguousarray(
            Wo[h * 512:(h + 1) * 512, :].reshape(GROUP, 128, HIDDEN)
        ).astype(bf)
        bqh = np.ascontiguousarray(
            bq[h * 512:(h + 1) * 512].reshape(GROUP, 128).T
        )
        in_maps.append({
            "xT": xTh, "wq": wqh, "wk": wkh, "wv": wvh, "wo": woh,
            "bq": bqh,
        })

    res = run_bass_kernel_spmd(
        nc, in_maps, list(range(8)), trace=TRACE, tmpdir=TMPDIR
    )
    LAST_RESULTS = res

    # host-side constant bias: (bv repeated per head group) @ Wo + bo
    bv_rep = np.broadcast_to(
        bv.reshape(NKV, 1, D), (NKV, GROUP, D)
    ).reshape(HIDDEN)
    bias_row = bv_rep @ Wo + bo  # [HIDDEN]

    out = np.empty((B, S, HIDDEN), np.float32)
    for b in range(B):
        acc = res.results[b * NKV + 0]["out"].astype(np.float32)
        for h in range(1, NKV):
            acc = acc + res.results[b * NKV + h]["out"].astype(np.float32)
        out[b] = acc + bias_row
    return out


# revision 7
# speedup vs baseline: 1.3321x; 1.0869x over previous
"""GQA attention kernel for 8 Trainium2 NeuronCores (v2).

Sharding: core c = 4*b + h handles batch b (of 2) and kv-head h (of 4),
i.e. one kv head + its 4 grouped query heads. Each core computes its head
group's partial contribution to the output projection; the host sums the
4 partials per batch. No collectives.

v2 changes vs v1 (461us):
  - all inputs bf16 (halves DMA, full-rate matmuls), BAND=512 projections
  - softmax denominator via vector accumulation of P tiles + gpsimd
    partition_all_reduce -- no PE den matmuls (-18% PE stream), no slow
    [1,512] reciprocal (3.3us each)
  - attention inner loop software-pipelined: scores(sk+1) issued between
    PV(sk) matmuls so the exp latency never stalls the PE
  - QTILE=1024 (fewer, longer instruction groups), out-projection of
    qtile 0 interleaved between attention g-blocks of qtile 1
  - output partials in bf16 (halves output DMA)

Device math per core (S=2048, H=2048, d=128):
  QT_g = (x @ Wq_g + bq_g)^T          [d, S]   g=0..3   (bf16 matmuls)
  KT   = (x @ Wk_h)^T                 [d, S]            (bk cancels in softmax)
  V    = x @ Wv_h                     [S, d]   (V^T then PE-transposed)
  S^T  = KT^T-blocks @ QT             [Sk, Sq]
  P^T  = exp(SCALE * S^T)             (bf16, no max-subtraction: |s| <~ 5)
  y^T  = V^T-blocks.T @ P^T (PSUM accum); den = partition_all_reduce(sum P^T)
  yT  := y^T * recip(den)
  out += yT_g^T @ Wo_g                [S, H]  partial, bf16 to HBM
Host: out[b] = sum_h partial + (bv_rep @ Wo + bo).
"""

import numpy as np
import ml_dtypes

B = 2
S = 2048
HIDDEN = 2048
NKV = 4
GROUP = 4
D = 128
SCALE = D ** -0.5

BAND = 512            # S-columns per projection band
NBAND = S // BAND     # 4
NCH = HIDDEN // 128   # 16 contraction chunks
QT2 = 1024            # queries per attention tile
NQT2 = S // QT2       # 2
NSK = S // 128        # 16 key tiles

_CACHE = {}
LAST_RESULTS = None
TRACE = False
TMPDIR = None


def _build():
    import concourse.bass as bass
    import concourse.bacc as bacc
    import concourse.mybir as mybir
    import concourse.tile as tile
    import concourse.bass_isa as bass_isa
    from concourse.masks import make_identity

    f32 = mybir.dt.float32
    bf16 = mybir.dt.bfloat16
    EXP = mybir.ActivationFunctionType.Exp
    IDENT = mybir.ActivationFunctionType.Identity
    COPY = mybir.ActivationFunctionType.Copy

    nc = bacc.Bacc(trn_type="TRN2", target_bir_lowering=False, debug=False)

    xT = nc.dram_tensor("xT", [NBAND, 128, NCH, BAND], bf16, kind="ExternalInput").ap()
    wq = nc.dram_tensor("wq", [GROUP, 128, NCH, 128], bf16, kind="ExternalInput").ap()
    wk = nc.dram_tensor("wk", [128, NCH, 128], bf16, kind="ExternalInput").ap()
    wv = nc.dram_tensor("wv", [128, NCH, 128], bf16, kind="ExternalInput").ap()
    wo = nc.dram_tensor("wo", [GROUP, 128, HIDDEN], bf16, kind="ExternalInput").ap()
    bq = nc.dram_tensor("bq", [128, GROUP], f32, kind="ExternalInput").ap()
    out = nc.dram_tensor("out", [S, HIDDEN], bf16, kind="ExternalOutput").ap()

    with tile.TileContext(nc) as tc:
        with (
            tc.tile_pool(name="const", bufs=1) as constp,
            tc.tile_pool(name="wts", bufs=1) as wtsp,
            tc.tile_pool(name="xb", bufs=2) as xbp,
            tc.tile_pool(name="qkv", bufs=1) as qkvp,
            tc.tile_pool(name="ptbuf", bufs=4) as ptp,
            tc.tile_pool(name="accb", bufs=2) as accp,
            tc.tile_pool(name="dens", bufs=2) as densp,
            tc.tile_pool(name="ytbuf", bufs=8) as ytp,
            tc.tile_pool(name="outbuf", bufs=2) as outp,
        ):
            # ---- DMAs in consumption order: consts, wk, band0, wv, wq ----
            bq_t = constp.tile([128, GROUP], f32, name="bq_t")
            nc.sync.dma_start(out=bq_t[:, :], in_=bq)
            ident = constp.tile([128, 128], f32, name="ident")
            make_identity(nc, ident[:, :])

            # wk and band 0 split into chunk-groups so the first K matmuls
            # can start before the whole tensors land
            wk_t = wtsp.tile([128, NCH, 128], bf16, name="wk_t")
            nc.sync.dma_start(out=wk_t[:, 0:4, :], in_=wk[:, 0:4, :])
            bands = [None] * NBAND
            bands[0] = xbp.tile([128, NCH, BAND], bf16, name="band", tag="band")
            nc.sync.dma_start(out=bands[0][:, 0:4, :], in_=xT[0, :, 0:4, :])
            nc.sync.dma_start(out=wk_t[:, 4:16, :], in_=wk[:, 4:16, :])
            for cq in range(1, 4):
                nc.sync.dma_start(
                    out=bands[0][:, 4 * cq:4 * (cq + 1), :],
                    in_=xT[0, :, 4 * cq:4 * (cq + 1), :],
                )

            wv_t = wtsp.tile([128, NCH, 128], bf16, name="wv_t")
            nc.sync.dma_start(out=wv_t[:, :, :], in_=wv)
            wq_t = []
            for g in range(GROUP):
                t = wtsp.tile([128, NCH, 128], bf16, name=f"wq_t{g}", tag=f"wq{g}")
                nc.sync.dma_start(out=t[:, :, :], in_=wq[g])
                wq_t.append(t)

            # ---- persistent activations ----
            qt_t = []
            for g in range(GROUP):
                t = qkvp.tile([128, S], bf16, name=f"qt{g}", tag=f"qt{g}")
                qt_t.append(t)
            kt_t = qkvp.tile([128, S], bf16, name="kt_t")
            v_t = qkvp.tile([128, NSK * 128], bf16, name="v_t")
            vt_f = qkvp.tile([128, S], f32, name="vt_f")

            # =============== phase 1: projections ===============
            with tc.tile_pool(name="psA", bufs=1, space="PSUM") as psA:
                for b in range(NBAND):
                    if bands[b] is None:
                        bands[b] = xbp.tile(
                            [128, NCH, BAND], bf16, name="band", tag="band"
                        )
                        nc.sync.dma_start(out=bands[b][:, :, :], in_=xT[b])
                    band = bands[b]
                    bsl = slice(b * BAND, (b + 1) * BAND)

                    # K^T accumulation
                    pk = psA.tile([128, BAND], f32, name="pk", tag="pacc", bufs=4)
                    for c in range(NCH):
                        nc.tensor.matmul(
                            out=pk[:, :],
                            lhsT=wk_t[:, c, :],
                            rhs=band[:, c, :],
                            start=(c == 0), stop=(c == NCH - 1),
                        )
                    nc.scalar.activation(kt_t[:, bsl], pk[:, :], COPY)

                    # V^T accumulation (f32, transposed to V per 128-block later)
                    pv = psA.tile([128, BAND], f32, name="pv", tag="pacc", bufs=4)
                    for c in range(NCH):
                        nc.tensor.matmul(
                            out=pv[:, :],
                            lhsT=wv_t[:, c, :],
                            rhs=band[:, c, :],
                            start=(c == 0), stop=(c == NCH - 1),
                        )
                    nc.scalar.activation(vt_f[:, bsl], pv[:, :], COPY)

                    # Q^T per local head
                    for g in range(GROUP):
                        pq = psA.tile([128, BAND], f32, name="pq", tag="pacc", bufs=4)
                        for c in range(NCH):
                            nc.tensor.matmul(
                                out=pq[:, :],
                                lhsT=wq_t[g][:, c, :],
                                rhs=band[:, c, :],
                                start=(c == 0), stop=(c == NCH - 1),
                            )
                        nc.scalar.activation(
                            qt_t[g][:, bsl], pq[:, :], IDENT,
                            bias=bq_t[:, g:g + 1],
                        )

                    # transpose V^T band -> V (4 sk-tiles per band)
                    for t in range(BAND // 128):
                        sk = b * (BAND // 128) + t
                        pt = psA.tile([128, 128], f32, name="ptr", tag="pacc", bufs=4)
                        nc.tensor.transpose(
                            pt[:, :], vt_f[:, sk * 128:(sk + 1) * 128], ident[:, :]
                        )
                        nc.scalar.activation(
                            v_t[:, sk * 128:(sk + 1) * 128], pt[:, :], COPY
                        )

            # wo loads (needed only by out-projection, off the critical path)
            wo_t = []
            for g in range(GROUP):
                t = wtsp.tile([128, HIDDEN], bf16, name=f"wo_t{g}", tag=f"wo{g}")
                nc.sync.dma_start(out=t[:, :], in_=wo[g])
                wo_t.append(t)

            # =============== phase 2+3: attention + out-projection ===============
            # PSUM budget (8 banks): tag "pp" [128,1024] bufs=2 (4 banks),
            # shared by score pairs and out-proj pairs (never live at once);
            # tag "py" [128,1024] bufs=2 (4 banks).
            with tc.tile_pool(name="psB", bufs=1, space="PSUM") as psB:
                yt_all = {}

                def attn_block(qt2, g):
                    """Attention for one head g over query tile qt2 (1024 q).

                    Inner loop software-pipelined: scores for sk+1 are issued
                    between the PV matmuls of sk so the PE never waits on exp.
                    One [128,1024] exp per sk (half the scalar-engine overhead
                    of two 512-wide ones).
                    """
                    q0 = qt2 * QT2
                    py = psB.tile([128, QT2], f32, name="py", tag="py", bufs=2)
                    acc = accp.tile([128, QT2], bf16, name="acc", tag="acc")
                    pts = [None] * NSK
                    pss = [None] * NSK

                    def scores(sk, h):
                        if h == 0:
                            pss[sk] = psB.tile(
                                [128, QT2], f32, name="ps", tag="pp", bufs=2
                            )
                        nc.tensor.matmul(
                            out=pss[sk][:, h * 512:(h + 1) * 512],
                            lhsT=kt_t[:, sk * 128:(sk + 1) * 128],
                            rhs=qt_t[g][:, q0 + h * 512: q0 + (h + 1) * 512],
                            start=True, stop=True,
                        )
                        if h == 1:
                            pts[sk] = ptp.tile([128, QT2], bf16, name="pt", tag="pt")
                            nc.scalar.activation(
                                pts[sk][:, :], pss[sk][:, :], EXP, scale=SCALE,
                            )

                    scores(0, 0)
                    scores(0, 1)
                    for sk in range(NSK):
                        if sk + 1 < NSK:
                            scores(sk + 1, 0)
                            scores(sk + 1, 1)
                        for h in range(2):
                            nc.tensor.matmul(
                                out=py[:, h * 512:(h + 1) * 512],
                                lhsT=v_t[:, sk * 128:(sk + 1) * 128],
                                rhs=pts[sk][:, h * 512:(h + 1) * 512],
                                start=(sk == 0), stop=(sk == NSK - 1),
                            )
                        if sk == 0:
                            # init copy on scalar: vector COPY is slow (2.6us)
                            # and would delay the add chain
                            nc.scalar.activation(acc[:, :], pts[sk][:, :], COPY)
                        else:
                            nc.vector.tensor_add(acc[:, :], acc[:, :], pts[sk][:, :])

                    # denominator: all-partition sum of acc (bf16 in, f32 out),
                    # fast approximate reciprocal, then scale y^T
                    bden = densp.tile([128, QT2], f32, name="bden", tag="bden")
                    nc.gpsimd.partition_all_reduce(
                        bden[:, :], acc[:, :], channels=128,
                        reduce_op=bass_isa.ReduceOp.add,
                    )
                    brecip = densp.tile([128, QT2], f32, name="brecip", tag="brecip")
                    nc.vector.reciprocal_approx_fast(brecip[:, :], bden[:, :])
                    yt = ytp.tile([128, QT2], bf16, name="yt", tag="yt")
                    nc.vector.tensor_mul(yt[:, :], py[:, :], brecip[:, :])
                    yt_all[(qt2, g)] = yt

                def outproj_iblocks(qt2, iblocks):
                    for i in iblocks:
                        outs = outp.tile([128, HIDDEN], bf16, name="outs", tag="outs")
                        for jp in range(2):
                            po = psB.tile([128, QT2], f32, name="po", tag="pp", bufs=2)
                            for jj in range(2):
                                j = jp * 2 + jj
                                for g in range(GROUP):
                                    nc.tensor.matmul(
                                        out=po[:, jj * 512:(jj + 1) * 512],
                                        lhsT=yt_all[(qt2, g)][:, i * 128:(i + 1) * 128],
                                        rhs=wo_t[g][:, j * 512:(j + 1) * 512],
                                        start=(g == 0), stop=(g == GROUP - 1),
                                    )
                            # drains on scalar only: on vector they queue behind
                            # the normalize chain (all_reduce -> recip -> mul)
                            # and stall the next po allocation for ~6us
                            nc.scalar.activation(
                                outs[:, jp * 1024:(jp + 1) * 1024], po[:, :], COPY
                            )
                        r0 = qt2 * QT2 + i * 128
                        nc.sync.dma_start(out=out[r0:r0 + 128, :], in_=outs[:, :])

                for g in range(GROUP):
                    attn_block(0, g)
                # out-proj of qtile 0 spread between qtile-1 attention blocks
                for g in range(GROUP):
                    attn_block(1, g)
                    outproj_iblocks(0, [2 * g, 2 * g + 1])
                outproj_iblocks(1, list(range(8)))

    nc.finalize()
    return nc


def _get_nc():
    if "nc" not in _CACHE:
        _CACHE["nc"] = _build()
    return _CACHE["nc"]


def kernel(x, Wq, bq, Wk, bk, Wv, bv, Wo, bo):
    global LAST_RESULTS
    from concourse.bass_utils import run_bass_kernel_spmd

    bf = ml_dtypes.bfloat16
    x = np.asarray(x, np.float32)
    Wq = np.asarray(Wq, np.float32)
    Wk = np.asarray(Wk, np.float32)
    Wv = np.asarray(Wv, np.float32)
    Wo = np.asarray(Wo, np.float32)
    bq = np.asarray(bq, np.float32)
    bv = np.asarray(bv, np.float32)
    bo = np.asarray(bo, np.float32)

    nc = _get_nc()

    in_maps = []
    for c in range(8):
        b, h = divmod(c, NKV)
        xT = x[b].T  # [HIDDEN, S]
        xTh = np.ascontiguousarray(
            xT.reshape(NCH, 128, NBAND, BAND).transpose(2, 1, 0, 3)
        ).astype(bf)
        # wq[g]: [128, NCH, 128] per local head
        wqh = np.ascontiguousarray(
            Wq[:, h * 512:(h + 1) * 512]
            .reshape(NCH, 128, GROUP, 128).transpose(2, 1, 0, 3)
        ).astype(bf)
        wkh = np.ascontiguousarray(
            Wk[:, h * 128:(h + 1) * 128].reshape(NCH, 128, 128).transpose(1, 0, 2)
        ).astype(bf)
        wvh = np.ascontiguousarray(
            Wv[:, h * 128:(h + 1) * 128].reshape(NCH, 128, 128).transpose(1, 0, 2)
        ).astype(bf)
        woh = np.ascontiguousarray(
            Wo[h * 512:(h + 1) * 512, :].reshape(GROUP, 128, HIDDEN)
        ).astype(bf)
        bqh = np.ascontiguousarray(
            bq[h * 512:(h + 1) * 512].reshape(GROUP, 128).T
        )
        in_maps.append({
            "xT": xTh, "wq": wqh, "wk": wkh, "wv": wvh, "wo": woh,
            "bq": bqh,
        })

    res = run_bass_kernel_spmd(
        nc, in_maps, list(range(8)), trace=TRACE, tmpdir=TMPDIR
    )
    LAST_RESULTS = res

    # host-side constant bias: (bv repeated per head group) @ Wo + bo
    bv_rep = np.broadcast_to(
        bv.reshape(NKV, 1, D), (NKV, GROUP, D)
    ).reshape(HIDDEN)
    bias_row = bv_rep @ Wo + bo  # [HIDDEN]

    out = np.empty((B, S, HIDDEN), np.float32)
    for b in range(B):
        acc = res.results[b * NKV + 0]["out"].astype(np.float32)
        for h in range(1, NKV):
            acc = acc + res.results[b * NKV + h]["out"].astype(np.float32)
        out[b] = acc + bias_row
    return out
